# revision 2
# baseline (speedup 1.0000x reference)
"""Trainium2 Bass kernel for CALayer with top-k channel masking.

Computation (per batch item):
  y = mean(x, spatial)                    # [C]
  h = relu(w1 @ y + b1)                   # [C/R]
  a = sigmoid(w2 @ h + b2)                # [C]
  idx = sort(top_k(a, 128).indices)       # ascending channel ids
  out = a[idx, None, None] * x[idx]       # [128, H, W]

Strategy: data-parallel over batch (32 items -> 8 cores x 4). Everything
on-device per core:
  - x[b] loaded once to SBUF [128 part, 2 chunk, 4096 spatial]; means via one
    DVE reduce (1/HW folded into prepacked w1T).
  - MLP with tiny PE matmuls; ranking done on pre-sigmoid logits z (monotone
    => same selection as sigmoid, better numerics).
  - rank[c] = #{c': z[c'] > z[c]} via PE transpose-broadcast of z +
    DVE tensor_scalar(is_gt) with accum_out.
  - mask m = rank < K; output slot p = exclusive-prefix-sum(m) via matmul
    with strict-upper-triangular constant; unselected slots -> 512 (OOB).
  - spatial sums on ACT (activation Copy + accum_out); xs = x * sigmoid(z)
    per-partition on DVE (tensor_scalar hits the 2x fp32 perf mode).
  - one indirect SBUF->DRAM scatter per (batch, chunk) with bounds_check=K-1,
    oob_is_err=False: unselected channels are dropped at descriptor level, so
    HBM sees only the 128 selected rows. Each scatter targets its own output
    tensor (chunk slot ranges are disjoint since selected chunk-0 channels
    always precede chunk-1 channels in ascending order), so no two scatters
    share a WAW dependency; the host merges with an exact add over the
    zero-initialized buffers.
"""

from contextlib import ExitStack

import numpy as np

import concourse.bass as bass
import concourse.tile as tile
from concourse import bacc, mybir
from concourse.bass_utils import run_bass_kernel_spmd
from concourse.masks import make_identity

N_CORES = 8
B_FULL, C, H, W = 32, 256, 64, 64
NB = B_FULL // N_CORES  # batch items per core
HW = H * W
K = 128  # top-k
P = 128  # partitions
NCH = C // P  # channel chunks
R = 16  # reduction dim
OOB = 512.0  # out-of-bounds slot for unselected channels
F32 = mybir.dt.float32


def _body(ctx: ExitStack, tc: "tile.TileContext", x_d, outs_d, w1t_d, w2t_d, b1_d, b2_d, sut_d, ones_d):
    nc = tc.nc
    AF = mybir.ActivationFunctionType
    ALU = mybir.AluOpType

    cpool = ctx.enter_context(tc.tile_pool(name="const", bufs=1))
    xp = ctx.enter_context(tc.tile_pool(name="x", bufs=3))
    xsp = ctx.enter_context(tc.tile_pool(name="xs", bufs=2))
    sp = ctx.enter_context(tc.tile_pool(name="small", bufs=4))
    gp = ctx.enter_context(tc.tile_pool(name="g", bufs=2))
    pp = ctx.enter_context(tc.tile_pool(name="ps", bufs=2, space="PSUM"))
    zp = ctx.enter_context(tc.tile_pool(name="zrep", bufs=2, space="PSUM"))

    # constants / weights (replicated on every core); loaded on the ACT HWDGE
    # queue so they don't sit ahead of the big x loads on the sync FIFO
    w1t_sb = cpool.tile([P, NCH, R], F32)
    nc.scalar.dma_start(w1t_sb[:], w1t_d.ap().rearrange("(k p) r -> p k r", p=P))
    w2t_sb = cpool.tile([R, C], F32)
    nc.scalar.dma_start(w2t_sb[:], w2t_d.ap())
    b1_sb = cpool.tile([R, 1], F32)
    nc.scalar.dma_start(b1_sb[:], b1_d.ap())
    b2_sb = cpool.tile([P, NCH], F32)
    nc.scalar.dma_start(b2_sb[:], b2_d.ap().rearrange("k p -> p k"))
    sut_sb = cpool.tile([P, P], F32)
    nc.scalar.dma_start(sut_sb[:], sut_d.ap())
    ones_sb = cpool.tile([P, P], F32)
    nc.scalar.dma_start(ones_sb[:], ones_d.ap())
    ident_sb = cpool.tile([P, P], F32)
    make_identity(nc, ident_sb[:])

    trash = cpool.tile([P, HW], F32)  # throwaway write target for means-accum

    tiles = {}

    def stats(b):
        """load x[b], means, MLP, rank, mask -> attn weights a_sb and slots qi."""
        xt = xp.tile([P, NCH, HW], F32, tag="x")
        x_src = x_d.ap()[b].rearrange("(k p) f -> p k f", p=P)
        # last batch: half-chunk loads + split accumulation so its stats (the
        # kernel tail) complete sooner after the final bytes land
        nh = 2 if b == NB - 1 else 1
        HH = HW // nh
        y2 = sp.tile([P, NCH, 2], F32, tag="y")
        for k in range(NCH):
            for h in range(nh):
                hs = slice(h * HH, (h + 1) * HH)
                nc.sync.dma_start(xt[:, k, hs], x_src[:, k, hs])
                nc.scalar.activation(trash[:, hs], xt[:, k, hs], AF.Copy, accum_out=y2[:, k, h : h + 1])

        # h = relu(w1 @ y + b1); accumulate over chunk/half columns in PSUM
        ht_ps = pp.tile([R, 1], F32, tag="ht")
        for k in range(NCH):
            for h in range(nh):
                nc.tensor.matmul(ht_ps[:], lhsT=w1t_sb[:, k, :], rhs=y2[:, k, h : h + 1], start=(k == 0 and h == 0), stop=(k == NCH - 1 and h == nh - 1))
        ht_sb = sp.tile([R, 1], F32, tag="htsb")
        nc.scalar.activation(ht_sb[:], ht_ps[:], AF.Relu, bias=b1_sb[:])

        # z = w2 @ h; zb = z + b2 (ranking logit), a = sigmoid(z + b2) (scaling)
        z_ps = pp.tile([P, NCH], F32, tag="z")
        for k in range(NCH):
            nc.tensor.matmul(z_ps[:, k : k + 1], lhsT=w2t_sb[:, k * P : (k + 1) * P], rhs=ht_sb[:], start=True, stop=True)
        zb_sb = sp.tile([P, NCH], F32, tag="zb")
        nc.vector.tensor_tensor(out=zb_sb[:], in0=z_ps[:], in1=b2_sb[:], op=ALU.add)
        a_sb = sp.tile([P, NCH], F32, tag="a")
        for k in range(NCH):
            nc.scalar.activation(a_sb[:, k : k + 1], z_ps[:, k : k + 1], AF.Sigmoid, bias=b2_sb[:, k : k + 1])

        # replicate zb across partitions: zrep[p, c'] = zb[c']
        zrep_ps = zp.tile([P, C], F32, tag="zrep")
        for k in range(NCH):
            nc.tensor.transpose(zrep_ps[:, k * P : (k + 1) * P], in_=zb_sb[:, k : k + 1].to_broadcast([P, P]), identity=ident_sb[:])

        # rank[c] = #{c': zb[c'] > zb[c]} (compare + count fused via accum_out)
        rank = sp.tile([P, NCH], F32, tag="rank")
        for k in range(NCH):
            g = gp.tile([P, C], F32, tag="g")
            nc.vector.tensor_scalar(g[:], zrep_ps[:], zb_sb[:, k : k + 1], None, ALU.is_gt, ALU.add, accum_out=rank[:, k : k + 1])

        # mask; slots via prefix-sum matmul with the OOB term folded into the
        # constant (sut = strict-upper - OOB*I, so unselected rows come out at
        # prefix - OOB); a single fused add(+OOB) + int32 cast feeds the scatter
        m = sp.tile([P, NCH], F32, tag="m")
        nc.vector.tensor_scalar(m[:], rank[:], float(K) - 0.5, None, ALU.is_lt)
        p_ps = pp.tile([P, NCH], F32, tag="p")
        nc.tensor.matmul(p_ps[:, 0:1], lhsT=sut_sb[:], rhs=m[:, 0:1], start=True, stop=True)
        nc.tensor.matmul(p_ps[:, 1:2], lhsT=ones_sb[:], rhs=m[:, 0:1], start=True, stop=False)
        nc.tensor.matmul(p_ps[:, 1:2], lhsT=sut_sb[:], rhs=m[:, 1:2], start=False, stop=True)
        qi = sp.tile([P, NCH], mybir.dt.int32, tag="qi")
        nc.vector.tensor_scalar(qi[:], p_ps[:], OOB, None, ALU.add)
        tiles[b] = (xt, a_sb, qi)

    def emit(b):
        """scale x[b] by attn weight into xs, scatter selected rows to out[b]."""
        xt, a_sb, qi = tiles.pop(b)
        xs = xsp.tile([P, NCH, HW], F32, tag="xs")
        for k in range(NCH):
            if b == NB - 1 and k == 1:
                # tail: chunk 1 on ACT so it runs concurrently with chunk 0 on DVE
                nc.scalar.activation(xs[:, k, :], xt[:, k, :], AF.Copy, scale=a_sb[:, k : k + 1])
            else:
                nc.vector.tensor_scalar(xs[:, k, :], xt[:, k, :], a_sb[:, k : k + 1], None, ALU.mult)
            nc.gpsimd.indirect_dma_start(
                out=outs_d[b][k].ap(),
                out_offset=bass.IndirectOffsetOnAxis(ap=qi[:, k : k + 1], axis=0),
                in_=xs[:, k, :],
                in_offset=None,
                bounds_check=K - 1,
                oob_is_err=False,
            )

    # software-pipelined emission: stats run one batch ahead of scale/scatter
    stats(0)
    stats(1)
    emit(0)
    stats(2)
    emit(1)
    stats(3)
    emit(2)
    emit(3)


def build_nc():
    nc = bacc.Bacc("TRN2", target_bir_lowering=False, debug=False, num_devices=N_CORES)
    x_d = nc.dram_tensor("x", [NB, C, HW], F32, kind="ExternalInput")
    w1t_d = nc.dram_tensor("w1t", [C, R], F32, kind="ExternalInput")
    w2t_d = nc.dram_tensor("w2t", [R, C], F32, kind="ExternalInput")
    b1_d = nc.dram_tensor("b1", [R, 1], F32, kind="ExternalInput")
    b2_d = nc.dram_tensor("b2", [NCH, P], F32, kind="ExternalInput")
    sut_d = nc.dram_tensor("sut", [P, P], F32, kind="ExternalInput")
    ones_d = nc.dram_tensor("ones", [P, P], F32, kind="ExternalInput")
    outs_d = [[nc.dram_tensor(f"out{b}c{k}", [K, HW], F32, kind="ExternalOutput") for k in range(NCH)] for b in range(NB)]
    with tile.TileContext(nc) as tc:
        with ExitStack() as ctx:
            _body(ctx, tc, x_d, outs_d, w1t_d, w2t_d, b1_d, b2_d, sut_d, ones_d)
    nc.compile()
    return nc


def make_in_maps(x, w1, b1, w2, b2):
    """Per-core input dicts. x: [32, 256, 64, 64] f32."""
    w1t = np.ascontiguousarray(w1.T).astype(np.float32) / float(HW)  # [C, R], mean folded in
    w2t = np.ascontiguousarray(w2.T).astype(np.float32)  # [R, C]
    b1c = b1.astype(np.float32).reshape(R, 1)
    b2c = b2.astype(np.float32).reshape(NCH, P)
    sut = np.triu(np.ones((P, P), np.float32), k=1) - OOB * np.eye(P, dtype=np.float32)
    ones = np.ones((P, P), np.float32)
    xr = np.ascontiguousarray(x.astype(np.float32).reshape(B_FULL, C, HW))
    in_maps = []
    for i in range(N_CORES):
        in_maps.append(
            {
                "x": np.ascontiguousarray(xr[i * NB : (i + 1) * NB]),
                "w1t": w1t,
                "w2t": w2t,
                "b1": b1c,
                "b2": b2c,
                "sut": sut,
                "ones": ones,
            }
        )
    return in_maps


def _install_ntff_hook():
    """Bridge the missing antenv.axon_hooks module so run_bass_kernel_spmd
    trace=True can capture NTFF profiles via the axon PJRT .so."""
    import sys
    import types

    if "antenv.axon_hooks" in sys.modules:
        return
    try:
        if "/root/.axon_site" not in sys.path:
            sys.path.insert(0, "/root/.axon_site")
        # the .so's profile entrypoint returns -1 until the axon PJRT
        # client has run at least one execute in this interpreter
        import jax
        import jax.numpy as jnp

        jax.block_until_ready(jnp.zeros((2, 2)) + 1.0)
        from trn_agent_boot.trn_boot import _ntff_profile_via_ctypes

        hook = _ntff_profile_via_ctypes("/opt/axon/libaxon_pjrt.so")
        mod = types.ModuleType("antenv.axon_hooks")
        mod.get_axon_ntff_profile_hook = lambda: hook
        mod.set_axon_ntff_profile_hook = lambda h: None
        sys.modules["antenv.axon_hooks"] = mod
    except Exception as e:  # degrade to no tracing
        print("ntff hook install failed:", e)


_NC_CACHE = {}


def get_nc():
    if "nc" not in _NC_CACHE:
        _NC_CACHE["nc"] = build_nc()
    return _NC_CACHE["nc"]


def kernel(x, w1, b1, w2, b2, topk, _trace=False, **_ignored):
    assert int(topk) == K, f"kernel hardcodes topk={K}, got {topk}"
    assert x.shape == (B_FULL, C, H, W)
    nc = get_nc()
    if _trace:
        _install_ntff_hook()
    in_maps = make_in_maps(np.asarray(x), np.asarray(w1), np.asarray(b1), np.asarray(w2), np.asarray(b2))
    res = run_bass_kernel_spmd(nc, in_maps, core_ids=list(range(N_CORES)), trace=_trace)
    # chunk scatters write disjoint slot ranges of each batch's output into
    # separate zero-initialized tensors; merging them is an exact add
    outs = [
        np.stack([res.results[i][f"out{b}c0"] + res.results[i][f"out{b}c1"] for b in range(NB)]).reshape(NB, K, H, W)
        for i in range(N_CORES)
    ]
    full = np.concatenate(outs, axis=0).astype(np.float32)
    if _trace:
        return full, res
    return full



# revision 4
# speedup vs baseline: 1.5009x; 1.5009x over previous
"""Trainium2 Bass kernel for CALayer with top-k channel masking.

Computation (per batch item):
  y = mean(x, spatial)                    # [C]
  h = relu(w1 @ y + b1)                   # [C/R]
  a = sigmoid(w2 @ h + b2)                # [C]
  idx = sort(top_k(a, 128).indices)       # ascending channel ids
  out = a[idx, None, None] * x[idx]       # [128, H, W]

Strategy: data-parallel over batch (32 items -> 8 cores x 4), fp16 spatial
data end-to-end (memory-bound kernel; host casts x to fp16, device writes
fp16, host casts back -- verified numerically: selection margin 8.4x, rel
err 2.9e-4 vs the 2e-2 gate). Per core this halves HBM traffic to
8.4 MB read + 4.2 MB write (~35 us roofline at 358 GB/s).

  - all x chunk loads are queued upfront on the sync HWDGE ring so reads
    stream at line rate; the tiny packed const tensor rides the ACT ring.
  - spatial sums per (batch, chunk): chunk 0 on DVE (tensor_scalar accum),
    chunk 1 on ACT (activation Copy accum) so the two engines halve the
    reduction latency; 1/HW is folded into the prepacked w1T.
  - MLP with tiny PE matmuls; ranking done on pre-sigmoid logits z
    (monotone => same selection as sigmoid).
  - rank[c] = #{c': z[c'] > z[c]} via PE transpose-broadcast of z + DVE
    tensor_scalar(is_gt) with accum_out; mask m = rank < K; output slot
    p = exclusive-prefix-sum(m) via matmul with strict-upper-triangular
    constant; unselected slots -> +512 (OOB).
  - xs = x * sigmoid(z) per-partition on DVE in fp16 (4x perf mode); the
    tail chunk goes on ACT so both engines work the critical path.
  - one indirect SBUF->DRAM scatter per (batch, chunk) with
    bounds_check=K-1, oob_is_err=False: unselected channels are dropped at
    descriptor level, so HBM sees only the 128 selected fp16 rows. Each
    scatter targets its own output tensor (chunk slot ranges are disjoint),
    so no WAW dependency; the host merges with an exact add over the
    zero-initialized buffers and casts back to fp32.
"""

from contextlib import ExitStack

import numpy as np

import concourse.bass as bass
import concourse.tile as tile
from concourse import bacc, mybir
from concourse.bass_utils import run_bass_kernel_spmd
from concourse.masks import make_identity

N_CORES = 8
B_FULL, C, H, W = 32, 256, 64, 64
NB = B_FULL // N_CORES  # batch items per core
HW = H * W
K = 128  # top-k
P = 128  # partitions
NCH = C // P  # channel chunks
R = 16  # reduction dim
OOB = 512.0  # out-of-bounds slot offset for unselected channels
F32 = mybir.dt.float32
F16 = mybir.dt.float16

# packed const tensor column layout: [w1t (2*16) | w2t (256) | b1 (1) | b2 (2) | sut (128) | ones (128)]
C_W2 = NCH * R
C_B1 = C_W2 + C
C_B2 = C_B1 + 1
C_SUT = C_B2 + NCH
C_ONES = C_SUT + P
NCOLS = C_ONES + P


def _body(ctx: ExitStack, tc: "tile.TileContext", x_d, outs_d, consts_d):
    nc = tc.nc
    AF = mybir.ActivationFunctionType
    ALU = mybir.AluOpType

    cpool = ctx.enter_context(tc.tile_pool(name="const", bufs=1))
    xp = ctx.enter_context(tc.tile_pool(name="x", bufs=NB))
    xsp = ctx.enter_context(tc.tile_pool(name="xs", bufs=4))
    sp = ctx.enter_context(tc.tile_pool(name="small", bufs=4))
    gp = ctx.enter_context(tc.tile_pool(name="g", bufs=2))
    pp = ctx.enter_context(tc.tile_pool(name="ps", bufs=2, space="PSUM"))
    zp = ctx.enter_context(tc.tile_pool(name="zrep", bufs=2, space="PSUM"))

    cs = cpool.tile([P, NCOLS], F32)
    nc.scalar.dma_start(cs[:], consts_d.ap())
    ident_sb = cpool.tile([P, P], F32)
    make_identity(nc, ident_sb[:])

    trash_v = cpool.tile([P, HW], F16)  # throwaway write targets for sum-accum
    trash_a = cpool.tile([P, HW], F16)

    # all x loads upfront on the sync HWDGE ring (independent; stream at line
    # rate). last batch in half-chunks so its sums chase the final bytes.
    xts = []
    for b in range(NB):
        xt = xp.tile([P, NCH, HW], F16, tag="x")
        x_src = x_d.ap()[b].rearrange("(k p) f -> p k f", p=P)
        nh = 2 if b == NB - 1 else 1
        HH = HW // nh
        for k in range(NCH):
            for h in range(nh):
                hs = slice(h * HH, (h + 1) * HH)
                nc.sync.dma_start(xt[:, k, hs], x_src[:, k, hs])
        xts.append(xt)

    tiles = {}

    def stats(b):
        """spatial sums, MLP, rank, mask -> attn weights and slots qi."""
        xt = xts[b]
        nh = 2 if b == NB - 1 else 1
        HH = HW // nh
        y2 = sp.tile([P, NCH, 2], F32, tag="y")
        for k in range(NCH):
            for h in range(nh):
                hs = slice(h * HH, (h + 1) * HH)
                if k == 0:
                    nc.vector.tensor_scalar(trash_v[:, hs], xt[:, k, hs], 1.0, None, ALU.mult, ALU.add, accum_out=y2[:, k, h : h + 1])
                else:
                    nc.scalar.activation(trash_a[:, hs], xt[:, k, hs], AF.Copy, accum_out=y2[:, k, h : h + 1])

        # h = relu(w1 @ y + b1); accumulate over chunk/half columns in PSUM
        ht_ps = pp.tile([R, 1], F32, tag="ht")
        for k in range(NCH):
            for h in range(nh):
                nc.tensor.matmul(ht_ps[:], lhsT=cs[:, k * R : (k + 1) * R], rhs=y2[:, k, h : h + 1], start=(k == 0 and h == 0), stop=(k == NCH - 1 and h == nh - 1))
        ht_sb = sp.tile([R, 1], F32, tag="htsb")
        nc.scalar.activation(ht_sb[:], ht_ps[:], AF.Relu, bias=cs[0:R, C_B1 : C_B1 + 1])

        # z = w2 @ h; zb = z + b2 (ranking logit), a = sigmoid(z + b2)
        z_ps = pp.tile([P, NCH], F32, tag="z")
        for k in range(NCH):
            nc.tensor.matmul(z_ps[:, k : k + 1], lhsT=cs[0:R, C_W2 + k * P : C_W2 + (k + 1) * P], rhs=ht_sb[:], start=True, stop=True)
        zb_sb = sp.tile([P, NCH], F32, tag="zb")
        nc.vector.tensor_tensor(out=zb_sb[:], in0=z_ps[:], in1=cs[:, C_B2 : C_B2 + NCH], op=ALU.add)
        a_sb = sp.tile([P, NCH], F32, tag="a")
        for k in range(NCH):
            nc.scalar.activation(a_sb[:, k : k + 1], z_ps[:, k : k + 1], AF.Sigmoid, bias=cs[:, C_B2 + k : C_B2 + k + 1])

        # replicate zb across partitions: zrep[p, c'] = zb[c']
        zrep_ps = zp.tile([P, C], F32, tag="zrep")
        for k in range(NCH):
            nc.tensor.transpose(zrep_ps[:, k * P : (k + 1) * P], in_=zb_sb[:, k : k + 1].to_broadcast([P, P]), identity=ident_sb[:])

        # rank[c] = #{c': zb[c'] > zb[c]} (compare + count fused via accum_out)
        rank = sp.tile([P, NCH], F32, tag="rank")
        for k in range(NCH):
            g = gp.tile([P, C], F32, tag="g")
            nc.vector.tensor_scalar(g[:], zrep_ps[:], zb_sb[:, k : k + 1], None, ALU.is_gt, ALU.add, accum_out=rank[:, k : k + 1])

        # mask; slots via prefix-sum matmul (sut = strict-upper - OOB*I);
        # fused add(+OOB) + int32 cast feeds the scatter
        m = sp.tile([P, NCH], F32, tag="m")
        nc.vector.tensor_scalar(m[:], rank[:], float(K) - 0.5, None, ALU.is_lt)
        p_ps = pp.tile([P, NCH], F32, tag="p")
        nc.tensor.matmul(p_ps[:, 0:1], lhsT=cs[:, C_SUT : C_SUT + P], rhs=m[:, 0:1], start=True, stop=True)
        nc.tensor.matmul(p_ps[:, 1:2], lhsT=cs[:, C_ONES : C_ONES + P], rhs=m[:, 0:1], start=True, stop=False)
        nc.tensor.matmul(p_ps[:, 1:2], lhsT=cs[:, C_SUT : C_SUT + P], rhs=m[:, 1:2], start=False, stop=True)
        qi = sp.tile([P, NCH], mybir.dt.int32, tag="qi")
        nc.vector.tensor_scalar(qi[:], p_ps[:], OOB, None, ALU.add)
        tiles[b] = (xt, a_sb, qi)

    def emit(b):
        """scale x[b] by attn weight into fp16 xs, scatter selected rows."""
        xt, a_sb, qi = tiles.pop(b)
        for k in range(NCH):
            xs = xsp.tile([P, HW], F16, tag="xs")
            if b == NB - 1 and k == 1:
                # tail: chunk 1 on ACT so it runs concurrently with chunk 0 on DVE
                nc.scalar.activation(xs[:], xt[:, k, :], AF.Copy, scale=a_sb[:, k : k + 1])
            else:
                nc.vector.tensor_scalar(xs[:], xt[:, k, :], a_sb[:, k : k + 1], None, ALU.mult)
            nc.gpsimd.indirect_dma_start(
                out=outs_d[b][k].ap(),
                out_offset=bass.IndirectOffsetOnAxis(ap=qi[:, k : k + 1], axis=0),
                in_=xs[:],
                in_offset=None,
                bounds_check=K - 1,
                oob_is_err=False,
            )

    # software-pipelined emission: stats run one batch ahead of scale/scatter
    stats(0)
    stats(1)
    emit(0)
    stats(2)
    emit(1)
    stats(3)
    emit(2)
    emit(3)


def build_nc():
    nc = bacc.Bacc("TRN2", target_bir_lowering=False, debug=False, num_devices=N_CORES)
    x_d = nc.dram_tensor("x", [NB, C, HW], F16, kind="ExternalInput")
    consts_d = nc.dram_tensor("consts", [P, NCOLS], F32, kind="ExternalInput")
    outs_d = [[nc.dram_tensor(f"out{b}c{k}", [K, HW], F16, kind="ExternalOutput") for k in range(NCH)] for b in range(NB)]
    with tile.TileContext(nc) as tc:
        with ExitStack() as ctx:
            _body(ctx, tc, x_d, outs_d, consts_d)
    nc.compile()
    return nc


def make_in_maps(x, w1, b1, w2, b2):
    """Per-core input dicts. x: [32, 256, 64, 64] f32 -> fp16 on host."""
    consts = np.zeros((P, NCOLS), np.float32)
    w1t = np.ascontiguousarray(w1.T).astype(np.float32) / float(HW)  # [C, R], mean folded in
    for k in range(NCH):
        consts[:, k * R : (k + 1) * R] = w1t[k * P : (k + 1) * P]
    consts[0:R, C_W2 : C_W2 + C] = w2.T.astype(np.float32)
    consts[0:R, C_B1] = b1.astype(np.float32)
    consts[:, C_B2 : C_B2 + NCH] = b2.astype(np.float32).reshape(NCH, P).T
    consts[:, C_SUT : C_SUT + P] = np.triu(np.ones((P, P), np.float32), k=1) - OOB * np.eye(P, dtype=np.float32)
    consts[:, C_ONES : C_ONES + P] = 1.0
    xr = np.ascontiguousarray(x.astype(np.float32).reshape(B_FULL, C, HW)).astype(np.float16)
    in_maps = []
    for i in range(N_CORES):
        in_maps.append(
            {
                "x": np.ascontiguousarray(xr[i * NB : (i + 1) * NB]),
                "consts": consts,
            }
        )
    return in_maps


def _install_ntff_hook():
    """Bridge the missing antenv.axon_hooks module so run_bass_kernel_spmd
    trace=True can capture NTFF profiles via the axon PJRT .so."""
    import sys
    import types

    if "antenv.axon_hooks" in sys.modules:
        return
    try:
        if "/root/.axon_site" not in sys.path:
            sys.path.insert(0, "/root/.axon_site")
        # the .so's profile entrypoint returns -1 until the axon PJRT
        # client has run at least one execute in this interpreter
        import jax
        import jax.numpy as jnp

        jax.block_until_ready(jnp.zeros((2, 2)) + 1.0)
        from trn_agent_boot.trn_boot import _ntff_profile_via_ctypes

        hook = _ntff_profile_via_ctypes("/opt/axon/libaxon_pjrt.so")
        mod = types.ModuleType("antenv.axon_hooks")
        mod.get_axon_ntff_profile_hook = lambda: hook
        mod.set_axon_ntff_profile_hook = lambda h: None
        sys.modules["antenv.axon_hooks"] = mod
    except Exception as e:  # degrade to no tracing
        print("ntff hook install failed:", e)


_NC_CACHE = {}


def get_nc():
    if "nc" not in _NC_CACHE:
        _NC_CACHE["nc"] = build_nc()
    return _NC_CACHE["nc"]


def kernel(x, w1, b1, w2, b2, topk, _trace=False, **_ignored):
    assert int(topk) == K, f"kernel hardcodes topk={K}, got {topk}"
    assert x.shape == (B_FULL, C, H, W)
    nc = get_nc()
    if _trace:
        _install_ntff_hook()
    in_maps = make_in_maps(np.asarray(x), np.asarray(w1), np.asarray(b1), np.asarray(w2), np.asarray(b2))
    res = run_bass_kernel_spmd(nc, in_maps, core_ids=list(range(N_CORES)), trace=_trace)
    # chunk scatters write disjoint slot ranges of each batch's output into
    # separate zero-initialized tensors; merging them is an exact add
    outs = [
        np.stack(
            [res.results[i][f"out{b}c0"].astype(np.float32) + res.results[i][f"out{b}c1"].astype(np.float32) for b in range(NB)]
        ).reshape(NB, K, H, W)
        for i in range(N_CORES)
    ]
    full = np.concatenate(outs, axis=0).astype(np.float32)
    if _trace:
        return full, res
    return full


# revision 5
# speedup vs baseline: 1.5786x; 1.0518x over previous
"""Trainium2 Bass kernel for CALayer with top-k channel masking.

Computation (per batch item):
  y = mean(x, spatial)                    # [C]
  h = relu(w1 @ y + b1)                   # [C/R]
  a = sigmoid(w2 @ h + b2)                # [C]
  idx = sort(top_k(a, 128).indices)       # ascending channel ids
  out = a[idx, None, None] * x[idx]       # [128, H, W]

Strategy: data-parallel over batch (32 items -> 8 cores x 4), fp16 spatial
data end-to-end (memory-bound kernel; host casts x to fp16, device writes
fp16, host casts back -- verified numerically: selection margin >14x, rel
err ~3e-4 vs the 2e-2 gate). Per core this halves HBM traffic to
8.4 MB read + 4.2 MB write (~35 us roofline at 358 GB/s).

  - all x chunk loads are queued upfront on the sync HWDGE ring so reads
    stream at line rate; the tiny packed const tensor rides the ACT ring.
  - spatial sums split across DVE and ACT per chunk. DVE side uses a
    one-level fp16 pairwise-add tree (tensor_tensor at 2x mode) before the
    1x-rate accumulating reduce, cutting its cost ~25%; ACT runs plain
    Copy+accum at 1x. 1/HW is folded into the prepacked w1T. The last
    batch is loaded and reduced in half-chunks so its sums chase the final
    bytes; the very last piece rides the DVE tree (shortest latency).
  - a dummy 1x1 sigmoid issued at kernel start pulls the ACT table load
    off the first batch's critical path (Copy/Relu ride the same set).
  - MLP with tiny PE matmuls on pre-sigmoid logits z (monotone => same
    selection as sigmoid); rank[c] = #{c': z[c'] > z[c]} via PE
    transpose-broadcast + DVE is_gt with accum_out; mask m = rank < K;
    slot = exclusive-prefix-sum(m) via matmul with strict-upper constant;
    unselected slots -> +512 (OOB).
  - xs = x * sigmoid(z) on DVE fp16 (4x perf mode, 1.3 us/chunk); two
    mid-kernel chunks go on ACT to balance engine load.
  - one indirect SBUF->DRAM scatter per (batch, chunk), bounds_check=K-1,
    oob_is_err=False: unselected channels are dropped at descriptor level
    so HBM sees only the 128 selected fp16 rows. Chunk scatters write
    disjoint slot ranges into separate zero-initialized tensors; the host
    merges with an exact add and casts back to fp32.
"""

from contextlib import ExitStack

import numpy as np

import concourse.bass as bass
import concourse.tile as tile
from concourse import bacc, mybir
from concourse.bass_utils import run_bass_kernel_spmd
from concourse.masks import make_identity

N_CORES = 8
B_FULL, C, H, W = 32, 256, 64, 64
NB = B_FULL // N_CORES  # batch items per core
HW = H * W
K = 128  # top-k
P = 128  # partitions
NCH = C // P  # channel chunks
R = 16  # reduction dim
OOB = 512.0  # out-of-bounds slot offset for unselected channels
F32 = mybir.dt.float32
F16 = mybir.dt.float16

# packed const tensor column layout: [w1t (2*16) | w2t (256) | b1 (1) | b2 (2) | sut (128) | ones (128)]
C_W2 = NCH * R
C_B1 = C_W2 + C
C_B2 = C_B1 + 1
C_SUT = C_B2 + NCH
C_ONES = C_SUT + P
NCOLS = C_ONES + P


def _body(ctx: ExitStack, tc: "tile.TileContext", x_d, outs_d, consts_d):
    nc = tc.nc
    AF = mybir.ActivationFunctionType
    ALU = mybir.AluOpType

    cpool = ctx.enter_context(tc.tile_pool(name="const", bufs=1))
    xp = ctx.enter_context(tc.tile_pool(name="x", bufs=NB))
    xsp = ctx.enter_context(tc.tile_pool(name="xs", bufs=4))
    tp = ctx.enter_context(tc.tile_pool(name="t1", bufs=2))
    sp = ctx.enter_context(tc.tile_pool(name="small", bufs=4))
    gp = ctx.enter_context(tc.tile_pool(name="g", bufs=2))
    pp = ctx.enter_context(tc.tile_pool(name="ps", bufs=2, space="PSUM"))
    zp = ctx.enter_context(tc.tile_pool(name="zrep", bufs=2, space="PSUM"))

    cs = cpool.tile([P, NCOLS], F32)
    nc.scalar.dma_start(cs[:], consts_d.ap())
    dum = cpool.tile([1, 2], F32)
    nc.scalar.activation(dum[0:1, 1:2], dum[0:1, 0:1], AF.Sigmoid)  # preload ACT table set
    ident_sb = cpool.tile([P, P], F32)
    make_identity(nc, ident_sb[:])

    trash_v = cpool.tile([P, HW // 2], F16)  # throwaway write targets for sum-accum
    trash_a = cpool.tile([P, HW], F16)

    # all x loads upfront on the sync HWDGE ring (independent; stream at line
    # rate). last batch in half-chunks so its sums chase the final bytes.
    xts = []
    for b in range(NB):
        xt = xp.tile([P, NCH, HW], F16, tag="x")
        x_src = x_d.ap()[b].rearrange("(k p) f -> p k f", p=P)
        nh = 2 if b == NB - 1 else 1
        HH = HW // nh
        for k in range(NCH):
            for h in range(nh):
                hs = slice(h * HH, (h + 1) * HH)
                nc.sync.dma_start(xt[:, k, hs], x_src[:, k, hs])
        xts.append(xt)

    def red_dve(xcol, n, y2col):
        """fp16 pairwise-add tree level (2x mode) + accumulating reduce."""
        t1 = tp.tile([P, HW // 2], F16, tag="t1")
        nc.vector.tensor_tensor(out=t1[:, : n // 2], in0=xcol[:, : n // 2], in1=xcol[:, n // 2 :], op=ALU.add)
        nc.vector.tensor_scalar(trash_v[:, : n // 2], t1[:, : n // 2], 1.0, None, ALU.mult, ALU.add, accum_out=y2col)

    def red_act(xcol, n, y2col):
        nc.scalar.activation(trash_a[:, :n], xcol, AF.Copy, accum_out=y2col)

    y2s = {}

    def reduces(b):
        """spatial sums for batch b -> y2s[b]; DVE/ACT split per chunk."""
        xt = xts[b]
        y2 = sp.tile([P, NCH, 2], F32, tag="y")
        if b < NB - 1:
            red_dve(xt[:, 0, :], HW, y2[:, 0, 0:1])
            red_act(xt[:, 1, :], HW, y2[:, 1, 0:1])
        else:
            # tail: c0 halves on ACT, c1 halves on the (faster) DVE tree so
            # the final piece's reduce is the short one
            HH = HW // 2
            for h in range(2):
                red_act(xt[:, 0, h * HH : (h + 1) * HH], HH, y2[:, 0, h : h + 1])
            for h in range(2):
                red_dve(xt[:, 1, h * HH : (h + 1) * HH], HH, y2[:, 1, h : h + 1])
        y2s[b] = y2

    tiles = {}

    def mlp(b):
        """MLP, rank, mask -> attn weights a_sb and slots qi for batch b."""
        y2 = y2s.pop(b)
        nh = 2 if b == NB - 1 else 1
        ht_ps = pp.tile([R, 1], F32, tag="ht")
        for k in range(NCH):
            for h in range(nh):
                nc.tensor.matmul(ht_ps[:], lhsT=cs[:, k * R : (k + 1) * R], rhs=y2[:, k, h : h + 1], start=(k == 0 and h == 0), stop=(k == NCH - 1 and h == nh - 1))
        ht_sb = sp.tile([R, 1], F32, tag="htsb")
        nc.scalar.activation(ht_sb[:], ht_ps[:], AF.Relu, bias=cs[0:R, C_B1 : C_B1 + 1])

        # z = w2 @ h; zb = z + b2 (ranking logit), a = sigmoid(z + b2)
        z_ps = pp.tile([P, NCH], F32, tag="z")
        for k in range(NCH):
            nc.tensor.matmul(z_ps[:, k : k + 1], lhsT=cs[0:R, C_W2 + k * P : C_W2 + (k + 1) * P], rhs=ht_sb[:], start=True, stop=True)
        zb_sb = sp.tile([P, NCH], F32, tag="zb")
        nc.vector.tensor_tensor(out=zb_sb[:], in0=z_ps[:], in1=cs[:, C_B2 : C_B2 + NCH], op=ALU.add)
        a_sb = sp.tile([P, NCH], F32, tag="a")
        for k in range(NCH):
            nc.scalar.activation(a_sb[:, k : k + 1], z_ps[:, k : k + 1], AF.Sigmoid, bias=cs[:, C_B2 + k : C_B2 + k + 1])

        # replicate zb across partitions: zrep[p, c'] = zb[c']
        zrep_ps = zp.tile([P, C], F32, tag="zrep")
        for k in range(NCH):
            nc.tensor.transpose(zrep_ps[:, k * P : (k + 1) * P], in_=zb_sb[:, k : k + 1].to_broadcast([P, P]), identity=ident_sb[:])

        # rank[c] = #{c': zb[c'] > zb[c]} (compare + count fused via accum_out)
        rank = sp.tile([P, NCH], F32, tag="rank")
        for k in range(NCH):
            g = gp.tile([P, C], F32, tag="g")
            nc.vector.tensor_scalar(g[:], zrep_ps[:], zb_sb[:, k : k + 1], None, ALU.is_gt, ALU.add, accum_out=rank[:, k : k + 1])

        # mask; slots via prefix-sum matmul (sut = strict-upper - OOB*I);
        # fused add(+OOB) + int32 cast feeds the scatter
        m = sp.tile([P, NCH], F32, tag="m")
        nc.vector.tensor_scalar(m[:], rank[:], float(K) - 0.5, None, ALU.is_lt)
        p_ps = pp.tile([P, NCH], F32, tag="p")
        nc.tensor.matmul(p_ps[:, 0:1], lhsT=cs[:, C_SUT : C_SUT + P], rhs=m[:, 0:1], start=True, stop=True)
        nc.tensor.matmul(p_ps[:, 1:2], lhsT=cs[:, C_ONES : C_ONES + P], rhs=m[:, 0:1], start=True, stop=False)
        nc.tensor.matmul(p_ps[:, 1:2], lhsT=cs[:, C_SUT : C_SUT + P], rhs=m[:, 1:2], start=False, stop=True)
        qi = sp.tile([P, NCH], mybir.dt.int32, tag="qi")
        nc.vector.tensor_scalar(qi[:], p_ps[:], OOB, None, ALU.add)
        tiles[b] = (xts[b], a_sb, qi)

    def emit(b, act_chunks=()):
        """scale x[b] by attn weight into fp16 xs, scatter selected rows."""
        xt, a_sb, qi = tiles.pop(b)
        for k in range(NCH):
            xs = xsp.tile([P, HW], F16, tag="xs")
            if k in act_chunks:
                nc.scalar.activation(xs[:], xt[:, k, :], AF.Copy, scale=a_sb[:, k : k + 1])
            else:
                nc.vector.tensor_scalar(xs[:], xt[:, k, :], a_sb[:, k : k + 1], None, ALU.mult)
            nc.gpsimd.indirect_dma_start(
                out=outs_d[b][k].ap(),
                out_offset=bass.IndirectOffsetOnAxis(ap=qi[:, k : k + 1], axis=0),
                in_=xs[:],
                in_offset=None,
                bounds_check=K - 1,
                oob_is_err=False,
            )

    # reduces run ahead (gated only by loads); MLP chains pipelined one
    # batch behind; scales/scatters chase. Two mid scales ride ACT.
    reduces(0)
    reduces(1)
    mlp(0)
    reduces(2)
    mlp(1)
    emit(0)
    reduces(3)
    mlp(2)
    emit(1, act_chunks=(1,))
    mlp(3)
    emit(2, act_chunks=(1,))
    emit(3)


def build_nc():
    nc = bacc.Bacc("TRN2", target_bir_lowering=False, debug=False, num_devices=N_CORES, enable_partition_id=False)
    x_d = nc.dram_tensor("x", [NB, C, HW], F16, kind="ExternalInput")
    consts_d = nc.dram_tensor("consts", [P, NCOLS], F32, kind="ExternalInput")
    outs_d = [[nc.dram_tensor(f"out{b}c{k}", [K, HW], F16, kind="ExternalOutput") for k in range(NCH)] for b in range(NB)]
    with tile.TileContext(nc) as tc:
        with ExitStack() as ctx:
            _body(ctx, tc, x_d, outs_d, consts_d)
    nc.compile()
    return nc


def make_in_maps(x, w1, b1, w2, b2):
    """Per-core input dicts. x: [32, 256, 64, 64] f32 -> fp16 on host."""
    consts = np.zeros((P, NCOLS), np.float32)
    w1t = np.ascontiguousarray(w1.T).astype(np.float32) / float(HW)  # [C, R], mean folded in
    for k in range(NCH):
        consts[:, k * R : (k + 1) * R] = w1t[k * P : (k + 1) * P]
    consts[0:R, C_W2 : C_W2 + C] = w2.T.astype(np.float32)
    consts[0:R, C_B1] = b1.astype(np.float32)
    consts[:, C_B2 : C_B2 + NCH] = b2.astype(np.float32).reshape(NCH, P).T
    consts[:, C_SUT : C_SUT + P] = np.triu(np.ones((P, P), np.float32), k=1) - OOB * np.eye(P, dtype=np.float32)
    consts[:, C_ONES : C_ONES + P] = 1.0
    xr = np.ascontiguousarray(x.astype(np.float32).reshape(B_FULL, C, HW)).astype(np.float16)
    in_maps = []
    for i in range(N_CORES):
        in_maps.append(
            {
                "x": np.ascontiguousarray(xr[i * NB : (i + 1) * NB]),
                "consts": consts,
            }
        )
    return in_maps


def _install_ntff_hook():
    """Bridge the missing antenv.axon_hooks module so run_bass_kernel_spmd
    trace=True can capture NTFF profiles via the axon PJRT .so."""
    import sys
    import types

    if "antenv.axon_hooks" in sys.modules:
        return
    try:
        if "/root/.axon_site" not in sys.path:
            sys.path.insert(0, "/root/.axon_site")
        # the .so's profile entrypoint returns -1 until the axon PJRT
        # client has run at least one execute in this interpreter
        import jax
        import jax.numpy as jnp

        jax.block_until_ready(jnp.zeros((2, 2)) + 1.0)
        from trn_agent_boot.trn_boot import _ntff_profile_via_ctypes

        hook = _ntff_profile_via_ctypes("/opt/axon/libaxon_pjrt.so")
        mod = types.ModuleType("antenv.axon_hooks")
        mod.get_axon_ntff_profile_hook = lambda: hook
        mod.set_axon_ntff_profile_hook = lambda h: None
        sys.modules["antenv.axon_hooks"] = mod
    except Exception as e:  # degrade to no tracing
        print("ntff hook install failed:", e)


_NC_CACHE = {}


def get_nc():
    if "nc" not in _NC_CACHE:
        _NC_CACHE["nc"] = build_nc()
    return _NC_CACHE["nc"]


def kernel(x, w1, b1, w2, b2, topk, _trace=False, **_ignored):
    assert int(topk) == K, f"kernel hardcodes topk={K}, got {topk}"
    assert x.shape == (B_FULL, C, H, W)
    nc = get_nc()
    if _trace:
        _install_ntff_hook()
    in_maps = make_in_maps(np.asarray(x), np.asarray(w1), np.asarray(b1), np.asarray(w2), np.asarray(b2))
    res = run_bass_kernel_spmd(nc, in_maps, core_ids=list(range(N_CORES)), trace=_trace)
    # chunk scatters write disjoint slot ranges of each batch's output into
    # separate zero-initialized tensors; merging them is an exact add
    outs = [
        np.stack(
            [res.results[i][f"out{b}c0"].astype(np.float32) + res.results[i][f"out{b}c1"].astype(np.float32) for b in range(NB)]
        ).reshape(NB, K, H, W)
        for i in range(N_CORES)
    ]
    full = np.concatenate(outs, axis=0).astype(np.float32)
    if _trace:
        return full, res
    return full


# revision 6
# speedup vs baseline: 1.7055x; 1.0804x over previous
"""Trainium2 Bass kernel for CALayer with top-k channel masking.

Computation (per batch item):
  y = mean(x, spatial)                    # [C]
  h = relu(w1 @ y + b1)                   # [C/R]
  a = sigmoid(w2 @ h + b2)                # [C]
  idx = sort(top_k(a, 128).indices)       # ascending channel ids
  out = a[idx, None, None] * x[idx]       # [128, H, W]

Strategy: data-parallel over batch (32 items -> 8 cores x 4), fp16 spatial
data end-to-end (memory-bound kernel; host casts x to fp16, device writes
fp16, host casts back). Selection stability and rel err (~3e-4 vs the 2e-2
gate) verified numerically, including the fp16 partial-sum trees (margin
>6x on the top-k boundary). Per core: 8.4 MB read + 4.2 MB write.

  - all x chunk loads are queued upfront on the sync HWDGE ring; the last
    batch is loaded in interleaved half-chunks so both reduce engines can
    chase the final bytes. The packed const tensor rides the ACT ring.
  - spatial sums: DVE chunks use a 3-level fp16 pairwise-add tree
    (tensor_tensor at 2x mode) before the 1x-rate accumulating reduce
    (~2.8 us/chunk vs 4.4 direct); ACT chunks run plain Copy+accum at 1x
    (3.7 us). All reduces are emitted under tc.high_priority() so the
    Tile scheduler never lets scales/smalls preempt a data-ready reduce
    (this inversion cost ~8 us in earlier revisions). 1/HW is folded into
    the prepacked w1T.
  - a dummy 1x1 sigmoid at kernel start pulls both ACT table loads off the
    first batch's critical path.
  - MLP with tiny PE matmuls on pre-sigmoid logits z (monotone => same
    selection as sigmoid); rank[c] = #{c': z[c'] > z[c]} via PE
    transpose-broadcast + DVE is_gt with accum_out; mask m = rank < K;
    slot = exclusive-prefix-sum(m) via matmul with strict-upper constant;
    unselected slots -> +512 (OOB).
  - xs = x * sigmoid(z) on DVE fp16 (4x perf mode, 1.3 us/chunk); b2c1's
    scale rides ACT in its natural post-reduce gap.
  - one indirect SBUF->DRAM scatter per (batch, chunk), bounds_check=K-1,
    oob_is_err=False: unselected channels are dropped at descriptor level
    so HBM sees only the 128 selected fp16 rows. Chunk scatters write
    disjoint slot ranges into separate zero-initialized tensors; the host
    merges with an exact add and casts back to fp32.
"""

from contextlib import ExitStack

import numpy as np

import concourse.bass as bass
import concourse.tile as tile
from concourse import bacc, mybir
from concourse.bass_utils import run_bass_kernel_spmd
from concourse.masks import make_identity

N_CORES = 8
B_FULL, C, H, W = 32, 256, 64, 64
NB = B_FULL // N_CORES  # batch items per core
HW = H * W
K = 128  # top-k
P = 128  # partitions
NCH = C // P  # channel chunks
R = 16  # reduction dim
OOB = 512.0  # out-of-bounds slot offset for unselected channels
F32 = mybir.dt.float32
F16 = mybir.dt.float16

# packed const tensor column layout: [w1t (2*16) | w2t (256) | b1 (1) | b2 (2) | sut (128) | ones (128)]
C_W2 = NCH * R
C_B1 = C_W2 + C
C_B2 = C_B1 + 1
C_SUT = C_B2 + NCH
C_ONES = C_SUT + P
NCOLS = C_ONES + P


def _body(ctx: ExitStack, tc: "tile.TileContext", x_d, outs_d, consts_d):
    nc = tc.nc
    AF = mybir.ActivationFunctionType
    ALU = mybir.AluOpType

    cpool = ctx.enter_context(tc.tile_pool(name="const", bufs=1))
    xp = ctx.enter_context(tc.tile_pool(name="x", bufs=NB))
    xsp = ctx.enter_context(tc.tile_pool(name="xs", bufs=4))
    tp = ctx.enter_context(tc.tile_pool(name="t1", bufs=2))
    t2p = ctx.enter_context(tc.tile_pool(name="t2", bufs=2))
    sp = ctx.enter_context(tc.tile_pool(name="small", bufs=4))
    gp = ctx.enter_context(tc.tile_pool(name="g", bufs=2))
    pp = ctx.enter_context(tc.tile_pool(name="ps", bufs=2, space="PSUM"))
    zp = ctx.enter_context(tc.tile_pool(name="zrep", bufs=2, space="PSUM"))

    cs = cpool.tile([P, NCOLS], F32)
    nc.scalar.dma_start(cs[:], consts_d.ap())
    dum = cpool.tile([1, 2], F32)
    nc.scalar.activation(dum[0:1, 1:2], dum[0:1, 0:1], AF.Sigmoid)  # preload ACT table set
    ident_sb = cpool.tile([P, P], F32)
    make_identity(nc, ident_sb[:])

    trash_v = cpool.tile([P, HW // 8], F16)  # throwaway write targets for sum-accum
    trash_a = cpool.tile([P, HW], F16)

    # all x loads upfront on the sync HWDGE ring (independent; stream at
    # line rate). last batch in interleaved half-chunks so both reduce
    # engines chase the final bytes.
    xts = []
    for b in range(NB):
        xt = xp.tile([P, NCH, HW], F16, tag="x")
        x_src = x_d.ap()[b].rearrange("(k p) f -> p k f", p=P)
        if b < NB - 1:
            for k in range(NCH):
                nc.sync.dma_start(xt[:, k, :], x_src[:, k, :])
        else:
            HH = HW // 2
            for h in range(2):
                for k in range(NCH):
                    hs = slice(h * HH, (h + 1) * HH)
                    nc.sync.dma_start(xt[:, k, hs], x_src[:, k, hs])
        xts.append(xt)

    def red_dve(xcol, n, y2col):
        """fp16 pairwise-add tree (tensor_tensor at 2x) + 1x accumulating
        reduce on the [P, n/8] tail. Partial-sum rounding verified safe."""
        t1 = tp.tile([P, HW // 2], F16, tag="t1")
        t2 = t2p.tile([P, HW // 4], F16, tag="t2")
        h = n // 2
        nc.vector.tensor_tensor(out=t1[:, :h], in0=xcol[:, :h], in1=xcol[:, h:], op=ALU.add)
        nc.vector.tensor_tensor(out=t2[:, : h // 2], in0=t1[:, : h // 2], in1=t1[:, h // 2 : h], op=ALU.add)
        nc.vector.tensor_tensor(out=t1[:, : h // 4], in0=t2[:, : h // 4], in1=t2[:, h // 4 : h // 2], op=ALU.add)
        nc.vector.tensor_scalar(trash_v[:, : h // 4], t1[:, : h // 4], 1.0, None, ALU.mult, ALU.add, accum_out=y2col)

    def red_act(xcol, n, y2col):
        nc.scalar.activation(trash_a[:, :n], xcol, AF.Copy, accum_out=y2col)

    y2s = {}

    def reduces(b):
        """spatial sums for batch b -> y2s[b]; DVE/ACT split per chunk.
        High priority: a data-ready reduce must never wait behind scales."""
        xt = xts[b]
        y2 = sp.tile([P, NCH, 2], F32, tag="y")
        with tc.high_priority():
            if b < NB - 1:
                red_dve(xt[:, 0, :], HW, y2[:, 0, 0:1])
                red_act(xt[:, 1, :], HW, y2[:, 1, 0:1])
            else:
                HH = HW // 2
                for h in range(2):
                    red_act(xt[:, 0, h * HH : (h + 1) * HH], HH, y2[:, 0, h : h + 1])
                    red_dve(xt[:, 1, h * HH : (h + 1) * HH], HH, y2[:, 1, h : h + 1])
        y2s[b] = y2

    tiles = {}

    def mlp(b):
        """MLP, rank, mask -> attn weights a_sb and slots qi for batch b."""
        y2 = y2s.pop(b)
        nh = 2 if b == NB - 1 else 1
        ht_ps = pp.tile([R, 1], F32, tag="ht")
        for k in range(NCH):
            for h in range(nh):
                nc.tensor.matmul(ht_ps[:], lhsT=cs[:, k * R : (k + 1) * R], rhs=y2[:, k, h : h + 1], start=(k == 0 and h == 0), stop=(k == NCH - 1 and h == nh - 1))
        ht_sb = sp.tile([R, 1], F32, tag="htsb")
        nc.scalar.activation(ht_sb[:], ht_ps[:], AF.Relu, bias=cs[0:R, C_B1 : C_B1 + 1])

        # z = w2 @ h; zb = z + b2 (ranking logit), a = sigmoid(z + b2)
        z_ps = pp.tile([P, NCH], F32, tag="z")
        for k in range(NCH):
            nc.tensor.matmul(z_ps[:, k : k + 1], lhsT=cs[0:R, C_W2 + k * P : C_W2 + (k + 1) * P], rhs=ht_sb[:], start=True, stop=True)
        zb_sb = sp.tile([P, NCH], F32, tag="zb")
        nc.vector.tensor_tensor(out=zb_sb[:], in0=z_ps[:], in1=cs[:, C_B2 : C_B2 + NCH], op=ALU.add)
        a_sb = sp.tile([P, NCH], F32, tag="a")
        for k in range(NCH):
            nc.scalar.activation(a_sb[:, k : k + 1], z_ps[:, k : k + 1], AF.Sigmoid, bias=cs[:, C_B2 + k : C_B2 + k + 1])

        # replicate zb across partitions: zrep[p, c'] = zb[c']
        zrep_ps = zp.tile([P, C], F32, tag="zrep")
        for k in range(NCH):
            nc.tensor.transpose(zrep_ps[:, k * P : (k + 1) * P], in_=zb_sb[:, k : k + 1].to_broadcast([P, P]), identity=ident_sb[:])

        # rank[c] = #{c': zb[c'] > zb[c]} (compare + count fused via accum_out)
        rank = sp.tile([P, NCH], F32, tag="rank")
        for k in range(NCH):
            g = gp.tile([P, C], F32, tag="g")
            nc.vector.tensor_scalar(g[:], zrep_ps[:], zb_sb[:, k : k + 1], None, ALU.is_gt, ALU.add, accum_out=rank[:, k : k + 1])

        # mask; slots via prefix-sum matmul (sut = strict-upper - OOB*I);
        # fused add(+OOB) + int32 cast feeds the scatter
        m = sp.tile([P, NCH], F32, tag="m")
        nc.vector.tensor_scalar(m[:], rank[:], float(K) - 0.5, None, ALU.is_lt)
        p_ps = pp.tile([P, NCH], F32, tag="p")
        nc.tensor.matmul(p_ps[:, 0:1], lhsT=cs[:, C_SUT : C_SUT + P], rhs=m[:, 0:1], start=True, stop=True)
        nc.tensor.matmul(p_ps[:, 1:2], lhsT=cs[:, C_ONES : C_ONES + P], rhs=m[:, 0:1], start=True, stop=False)
        nc.tensor.matmul(p_ps[:, 1:2], lhsT=cs[:, C_SUT : C_SUT + P], rhs=m[:, 1:2], start=False, stop=True)
        qi = sp.tile([P, NCH], mybir.dt.int32, tag="qi")
        nc.vector.tensor_scalar(qi[:], p_ps[:], OOB, None, ALU.add)
        tiles[b] = (xts[b], a_sb, qi)

    def emit(b, act_chunks=()):
        """scale x[b] by attn weight into fp16 xs, scatter selected rows."""
        xt, a_sb, qi = tiles.pop(b)
        for k in range(NCH):
            xs = xsp.tile([P, HW], F16, tag="xs")
            if k in act_chunks:
                nc.scalar.activation(xs[:], xt[:, k, :], AF.Copy, scale=a_sb[:, k : k + 1])
            else:
                nc.vector.tensor_scalar(xs[:], xt[:, k, :], a_sb[:, k : k + 1], None, ALU.mult)
            nc.gpsimd.indirect_dma_start(
                out=outs_d[b][k].ap(),
                out_offset=bass.IndirectOffsetOnAxis(ap=qi[:, k : k + 1], axis=0),
                in_=xs[:],
                in_offset=None,
                bounds_check=K - 1,
                oob_is_err=False,
            )

    # reduces run ahead (gated only by loads, and never preempted thanks to
    # high_priority); MLP chains pipelined one batch behind; scales and
    # scatters fill the gaps.
    reduces(0)
    reduces(1)
    mlp(0)
    reduces(2)
    mlp(1)
    emit(0)
    reduces(3)
    mlp(2)
    emit(1)
    mlp(3)
    emit(2, act_chunks=(1,))
    emit(3)


def build_nc():
    nc = bacc.Bacc("TRN2", target_bir_lowering=False, debug=False, num_devices=N_CORES, enable_partition_id=False)
    x_d = nc.dram_tensor("x", [NB, C, HW], F16, kind="ExternalInput")
    consts_d = nc.dram_tensor("consts", [P, NCOLS], F32, kind="ExternalInput")
    outs_d = [[nc.dram_tensor(f"out{b}c{k}", [K, HW], F16, kind="ExternalOutput") for k in range(NCH)] for b in range(NB)]
    with tile.TileContext(nc) as tc:
        with ExitStack() as ctx:
            _body(ctx, tc, x_d, outs_d, consts_d)
    nc.compile()
    return nc


def make_in_maps(x, w1, b1, w2, b2):
    """Per-core input dicts. x: [32, 256, 64, 64] f32 -> fp16 on host."""
    consts = np.zeros((P, NCOLS), np.float32)
    w1t = np.ascontiguousarray(w1.T).astype(np.float32) / float(HW)  # [C, R], mean folded in
    for k in range(NCH):
        consts[:, k * R : (k + 1) * R] = w1t[k * P : (k + 1) * P]
    consts[0:R, C_W2 : C_W2 + C] = w2.T.astype(np.float32)
    consts[0:R, C_B1] = b1.astype(np.float32)
    consts[:, C_B2 : C_B2 + NCH] = b2.astype(np.float32).reshape(NCH, P).T
    consts[:, C_SUT : C_SUT + P] = np.triu(np.ones((P, P), np.float32), k=1) - OOB * np.eye(P, dtype=np.float32)
    consts[:, C_ONES : C_ONES + P] = 1.0
    xr = np.ascontiguousarray(x.astype(np.float32).reshape(B_FULL, C, HW)).astype(np.float16)
    in_maps = []
    for i in range(N_CORES):
        in_maps.append(
            {
                "x": np.ascontiguousarray(xr[i * NB : (i + 1) * NB]),
                "consts": consts,
            }
        )
    return in_maps


def _install_ntff_hook():
    """Bridge the missing antenv.axon_hooks module so run_bass_kernel_spmd
    trace=True can capture NTFF profiles via the axon PJRT .so."""
    import sys
    import types

    if "antenv.axon_hooks" in sys.modules:
        return
    try:
        if "/root/.axon_site" not in sys.path:
            sys.path.insert(0, "/root/.axon_site")
        # the .so's profile entrypoint returns -1 until the axon PJRT
        # client has run at least one execute in this interpreter
        import jax
        import jax.numpy as jnp

        jax.block_until_ready(jnp.zeros((2, 2)) + 1.0)
        from trn_agent_boot.trn_boot import _ntff_profile_via_ctypes

        hook = _ntff_profile_via_ctypes("/opt/axon/libaxon_pjrt.so")
        mod = types.ModuleType("antenv.axon_hooks")
        mod.get_axon_ntff_profile_hook = lambda: hook
        mod.set_axon_ntff_profile_hook = lambda h: None
        sys.modules["antenv.axon_hooks"] = mod
    except Exception as e:  # degrade to no tracing
        print("ntff hook install failed:", e)


_NC_CACHE = {}


def get_nc():
    if "nc" not in _NC_CACHE:
        _NC_CACHE["nc"] = build_nc()
    return _NC_CACHE["nc"]


def kernel(x, w1, b1, w2, b2, topk, _trace=False, **_ignored):
    assert int(topk) == K, f"kernel hardcodes topk={K}, got {topk}"
    assert x.shape == (B_FULL, C, H, W)
    nc = get_nc()
    if _trace:
        _install_ntff_hook()
    in_maps = make_in_maps(np.asarray(x), np.asarray(w1), np.asarray(b1), np.asarray(w2), np.asarray(b2))
    res = run_bass_kernel_spmd(nc, in_maps, core_ids=list(range(N_CORES)), trace=_trace)
    # chunk scatters write disjoint slot ranges of each batch's output into
    # separate zero-initialized tensors; merging them is an exact add
    outs = [
        np.stack(
            [res.results[i][f"out{b}c0"].astype(np.float32) + res.results[i][f"out{b}c1"].astype(np.float32) for b in range(NB)]
        ).reshape(NB, K, H, W)
        for i in range(N_CORES)
    ]
    full = np.concatenate(outs, axis=0).astype(np.float32)
    if _trace:
        return full, res
    return full
